# revision 13
# baseline (speedup 1.0000x reference)
"""Bipartite matcher v4: fp8(e5m2) exp-code, pair-sorted packing, DoubleRow PE.

Device input per core: e8 = e5m2 code of 2^(KEXP*(x-1)/ln2)  [512, m_pad],
monotone nonnegative byte code of x (1 byte/elem -> DMA halves vs bf16).

Row side (DVE): reinterpret byte pairs as uint16. Lexicographic uint16 max of
packed pairs yields the exact max over EVEN columns in the high byte; a second
tree over (u<<8) yields the exact max over ODD columns. Both at 2x (2-byte TT).
Outputs rbmA/rbmB [128, n_chunks*nblk] uint16 per 512-column block.

Col side (PE): fp8e5 matmul against a shared block-diag ones stationary
[128, 32] (4-row slices), accumulated over the 4 row-chunks: colg group G =
sum of E over rows {4G..4G+4}+128c. Act copies PSUM->SBUF bf16.

Host recovery identical in structure to kernel2 (code-agnostic bounds):
  row: candidate blocks = blocks whose byte code max equals the row max
  col: groups with s_g >= s_max*0.99/16 contain the col argmax; all-zero
       columns (code underflow, ~5%) fall into the ncand>K full-scan path.
"""

import numpy as np

N = 512
M = 200000
NCORES = 8
M_SH = M // NCORES          # 25000
SUPER_W = 4096
M_PAD = 25088               # 6*4096 + 512
ROW_BLK = 512
NBLK = M_PAD // ROW_BLK     # 49
GRP_SL = 8                  # col-side row-slice height within a 128-chunk
NGRP = 128 // GRP_SL        # 16 groups; group G = rows {8G..8G+8} + 128c
GRP_ROWS = GRP_SL * 4       # 32 rows per group
PAD_VAL = 0.0
KEXP = 2000.0
EPS = np.float32(1e-12)

_CACHE: dict = {}


def _build_nc(m_pad=M_PAD, n_rows=N, loop_k=1):
    from concourse import bacc, mybir
    from concourse.tile import TileContext
    import concourse.bass as bass

    f8 = mybir.dt.float8e5
    u16 = mybir.dt.uint16
    bf16 = mybir.dt.bfloat16
    f32 = mybir.dt.float32
    n_chunks = n_rows // 128
    nblk = m_pad // ROW_BLK
    ngrp = NGRP

    nc = bacc.Bacc(None, target_bir_lowering=False)
    e_sh = nc.declare_dram_parameter("e_sh", [n_rows, m_pad // 2], f8, isOutput=False)
    wst = nc.declare_dram_parameter("wst", [128, 2 * ngrp], f8, isOutput=False)
    if loop_k > 1:
        nc.declare_dram_parameter("k_tag", [1, loop_k], f32, isOutput=False)
    rbma = nc.declare_dram_parameter("rbma", [128, n_chunks * nblk], u16, isOutput=True)
    rbmb = nc.declare_dram_parameter("rbmb", [128, n_chunks * nblk], u16, isOutput=True)
    colg = nc.declare_dram_parameter("colg", [ngrp, m_pad // 2], bf16, isOutput=True)

    tiles = []
    base = 0
    while base < m_pad:
        w = min(SUPER_W, m_pad - base)
        tiles.append((base, w))
        base += w
    # smallest supertile first: engines start after ~256KB instead of ~2MB
    tiles.sort(key=lambda t: t[1])

    with TileContext(nc) as tc:
        with (
            tc.tile_pool(name="x", bufs=8) as xpool,
            tc.tile_pool(name="lvl", bufs=2) as lpool,
            tc.tile_pool(name="cg", bufs=2) as cgpool,
            tc.tile_pool(name="outs", bufs=1) as opool,
            tc.tile_pool(name="ps", bufs=2, space=bass.MemorySpace.PSUM) as pspool,
        ):
            rbma_t = opool.tile([128, n_chunks * nblk], u16, name="rbmat", tag="rbmat")
            rbmb_t = opool.tile([128, n_chunks * nblk], u16, name="rbmbt", tag="rbmbt")
            wst_t = opool.tile([128, 2 * ngrp], f8, name="wst", tag="wst")
            nc.sync.dma_start(out=wst_t[:], in_=wst[:, :])

            def row_tree(src16, B, out_t, b0, nb):
                """uint16 max tree over pair-max bytes: j 128->64->32, reduce."""
                u1 = lpool.tile([128, B * 64], u16, name="v1", tag="v1")
                s3 = src16.rearrange("p (B j) -> p B j", j=128)
                nc.vector.tensor_tensor(
                    out=u1[:].rearrange("p (B j) -> p B j", j=64),
                    in0=s3[:, :, 0:64], in1=s3[:, :, 64:128],
                    op=mybir.AluOpType.max,
                )
                u2 = lpool.tile([128, B * 32], u16, name="v2", tag="v2")
                u13 = u1[:].rearrange("p (B j) -> p B j", j=64)
                nc.vector.tensor_tensor(
                    out=u2[:].rearrange("p (B j) -> p B j", j=32),
                    in0=u13[:, :, 0:32], in1=u13[:, :, 32:64],
                    op=mybir.AluOpType.max,
                )
                rb3 = out_t[:].rearrange("p (c b) -> p c b", b=nblk)
                nc.vector.tensor_reduce(
                    out=rb3[:, :, b0 // ROW_BLK:b0 // ROW_BLK + nb],
                    in_=u2[:].rearrange("p (B j) -> p B j", j=32),
                    axis=mybir.AxisListType.X,
                    op=mybir.AluOpType.max,
                )

            def body():
                for (b0, w) in tiles:
                    nb = w // ROW_BLK
                    B = n_chunks * nb
                    hw = w // 2             # pair-max bytes per supertile
                    xt = xpool.tile([128, n_chunks * hw], f8, name="xt", tag="x")
                    for c in range(n_chunks):
                        nc.sync.dma_start(
                            out=xt[:, c * hw:(c + 1) * hw],
                            in_=e_sh[c * 128:(c + 1) * 128, b0 // 2:(b0 + w) // 2],
                        )
                    x16 = xt[:].bitcast(mybir.dt.uint16)
                    # unsorted pair-max bytes: tree A hi = max over odd bytes,
                    # tree B (u<<8) hi = max over even bytes; host maxes both
                    row_tree(x16, B, rbma_t, b0, nb)
                    sh = lpool.tile([128, B * 128], u16, name="sh", tag="sh")
                    nc.vector.tensor_scalar(
                        out=sh[:], in0=x16, scalar1=8, scalar2=0,
                        op0=mybir.AluOpType.logical_shift_left,
                        op1=mybir.AluOpType.bitwise_or,
                    )
                    row_tree(sh[:], B, rbmb_t, b0, nb)
                    # ---- PE col-side group sums (fp8), Act copies to SBUF
                    # host decode only reads the pair-max (odd) columns, so
                    # the PE sums just those via a stride-2 moving AP: half
                    # the matmul cols, half the copies, half the colg out
                    HB = ROW_BLK // 2       # 256 odd columns per 512-block
                    cgt = cgpool.tile([ngrp, w // 2], bf16, name="cgt", tag="cg")
                    PSW = 4
                    for pg in range(0, nb, PSW):
                        bw = min(PSW, nb - pg)
                        ps = pspool.tile([ngrp, PSW * HB], f32, name="ps", tag="ps")
                        x4 = xt[:].rearrange("p (c q) -> p c q", c=n_chunks)
                        w3 = wst_t[:].rearrange("p (t g) -> p t g", t=2)
                        for blk in range(pg, pg + bw):
                            o = (blk - pg) * HB
                            for cp in range(n_chunks // 2):
                                nc.tensor.matmul(
                                    ps[:, o:o + HB],
                                    w3,
                                    x4[:, 2 * cp:2 * cp + 2,
                                       blk * HB:(blk + 1) * HB],
                                    start=(cp == 0),
                                    stop=(cp == n_chunks // 2 - 1),
                                    perf_mode=mybir.MatmulPerfMode.DoubleRow,
                                )
                        nc.scalar.copy(
                            out=cgt[:, pg * HB: pg * HB + bw * HB],
                            in_=ps[:, :bw * HB],
                        )
                    # outputs go via the idle GpSimd SWDGE queue so a copy-
                    # gated output DMA never head-of-line blocks input DMAs
                    nc.gpsimd.dma_start(
                        out=colg[:, b0 // 2:(b0 + w) // 2], in_=cgt[:]
                    )

            if loop_k == 1:
                body()
            else:
                with tc.For_i(0, loop_k, 1):
                    body()

            nc.gpsimd.dma_start(out=rbma[:, :], in_=rbma_t[:])
            nc.gpsimd.dma_start(out=rbmb[:, :], in_=rbmb_t[:])
    nc.compile()
    return nc


def _make_wst():
    import ml_dtypes

    w = np.zeros((128, 2, NGRP), np.float32)
    for p in range(128):
        w[p, :, p // GRP_SL] = 1.0
    return w.reshape(128, 2 * NGRP).astype(ml_dtypes.float8_e5m2)


def _group_rows(n_rows=N):
    g = np.arange(NGRP)
    rows = (
        g[:, None, None] * GRP_SL
        + np.arange(GRP_SL)[None, :, None]
        + 128 * np.arange(n_rows // 128)[None, None, :]
    )
    return np.sort(rows.reshape(NGRP, -1), axis=1).astype(np.int32)  # [32, 16]


def encode(x):
    """e5m2 byte code ~ 2^(KEXP*(x-1)/ln2), built directly in code space.

    The e5m2 bit pattern ((e+15)<<2)|m is a log-linear code, so
    p = 4*(KEXP*(x-1)/ln2 + 15) + 0.5, clamped to [0, 255] and truncated, is a
    monotone nonnegative code of x (Schraudolph trick at 8-bit width). The
    candidate bounds only need monotonicity + nonnegativity, not exactness.
    """
    import ml_dtypes

    a8 = np.float32(4.0 * KEXP / np.log(2.0))
    z = (x.astype(np.float32) - np.float32(1.0)) * a8 + np.float32(60.5)
    np.maximum(z, np.float32(0.0), out=z)
    e = z.astype(np.uint8)
    # ship only each adjacent column pair's max byte [N, M/2]: the row-side
    # tree's block max and the col-side pair-max sums depend on nothing else
    return np.maximum(e[:, 0::2], e[:, 1::2]).view(ml_dtypes.float8_e5m2)


def _get_nc():
    if "nc" not in _CACHE:
        _CACHE["nc"] = _build_nc()
    return _CACHE["nc"]


def _device_outputs(e_parts, wst):
    import os

    from concourse.bass_utils import run_bass_kernel_spmd

    in_maps = [{"e_sh": e_parts[c], "wst": wst} for c in range(NCORES)]
    try:
        bkr = run_bass_kernel_spmd(_get_nc(), in_maps, list(range(NCORES)))
    except ModuleNotFoundError:
        # profiling hook unavailable in this environment: run untraced
        os.environ["BASS_NEVER_TRACE"] = "1"
        bkr = run_bass_kernel_spmd(_get_nc(), in_maps, list(range(NCORES)))
    _CACHE["last_bkr"] = bkr
    res = bkr.results
    rbm_all = []
    colg_all = []
    for c in range(NCORES):
        ra = np.asarray(res[c]["rbma"]).view(np.uint16) >> np.uint16(8)
        rb = np.asarray(res[c]["rbmb"]).view(np.uint16) >> np.uint16(8)
        rbm_all.append(np.maximum(ra, rb).astype(np.uint8))
        colg_all.append(np.asarray(res[c]["colg"]))
    return rbm_all, colg_all


def _combine(x, rbm_all, colg_all, cand_k=4):
    import ml_dtypes

    n, m = x.shape

    # ---- row side ---------------------------------------------------------
    rbm_full = np.concatenate(
        [
            rbm_all[k].astype(np.int16).reshape(128, 4, NBLK)
            .transpose(1, 0, 2).reshape(n, NBLK)
            for k in range(NCORES)
        ],
        axis=1,
    )  # [512, 8*NBLK] byte codes
    rmax = rbm_full.max(axis=1)
    bp = np.empty(n, np.int64)
    for i in range(n):
        cand = np.flatnonzero(rbm_full[i] == rmax[i])
        segs, idxs = [], []
        for gb in cand:
            core, blk = divmod(int(gb), NBLK)
            c0 = blk * ROW_BLK
            w = min(ROW_BLK, M_SH - c0)
            if w <= 0:
                continue
            g0 = core * M_SH + c0
            segs.append(x[i, g0:g0 + w])
            idxs.append(np.arange(g0, g0 + w))
        if not segs:  # degenerate: whole-row code underflow
            bp[i] = int(x[i].argmax())
            continue
        vals = np.concatenate(segs)
        cols = np.concatenate(idxs)
        bp[i] = cols[int(vals.argmax())]

    # ---- col side: iterative exact decode at pair granularity -------------
    # Device col 2k+1 holds pair-max codes; S_hi[g,k] = sum of 16 pair-max
    # codes. Any row r with x[r,m] > cm has group sum S_hi >= code(x[r,m])
    # >= code(cm), so gathering all ungathered groups meeting that bound and
    # repeating until none remain yields the exact col max + first argmax.
    S_hi = np.concatenate(
        [colg_all[k][:, :M_SH // 2].astype(np.float32) for k in range(NCORES)],
        axis=1,
    )  # [16, M/2]: device emits pair-max-column sums only
    grows = _group_rows()                                   # [32, 16] int32
    a8 = np.float32(4.0 * KEXP / np.log(2.0))
    c605 = np.float32(60.5)

    mm = m
    colidx = np.arange(mm, dtype=np.int64)
    gathered = np.zeros((NGRP, mm // 2), bool)              # per PAIR
    cm = np.full(mm, -1.0, np.float32)
    ct = np.full(mm, 10**6, np.int64)

    # pass 0: top-1 group by S_hi per pair
    g0 = S_hi.argmax(axis=0)                                # [M/2]
    gathered[g0, np.arange(mm // 2)] = True
    rows0 = grows[np.repeat(g0, 2)]                         # [M, 16]
    sub0 = x[rows0.T, colidx[None, :]]                      # [16, M]
    cm = sub0.max(axis=0)
    ach = sub0 == cm[None, :]
    ct = np.where(ach, rows0.T, np.int64(10**6)).min(axis=0)

    lut = np.arange(256, dtype=np.uint8).view(ml_dtypes.float8_e5m2).astype(
        np.float32
    )  # byte code -> e5m2 value (the domain the PE sums live in)
    for _ in range(32):
        t = (cm - np.float32(1.0)) * a8 + c605
        np.maximum(t, np.float32(0.0), out=t)
        tcode = lut[t.astype(np.uint8)]                     # value of code(cm)
        tpair = np.minimum(tcode[0::2], tcode[1::2])        # conservative per pair
        # 0.99: bf16-stored sums can sit ~2^-9 below the true sum
        passing = (S_hi >= (tpair * np.float32(0.99))[None, :]) & ~gathered
        npass = passing.sum(axis=0)
        needp = np.flatnonzero(npass > 0)
        if needp.size == 0:
            break
        # gather up to 4 passing groups per needy pair this round
        sel = np.argsort(~passing[:, needp], axis=0, kind="stable")[:4]  # passing first
        selpass = np.take_along_axis(passing[:, needp], sel, axis=0)
        gathered[sel, needp[None, :]] |= selpass
        gsel = np.where(selpass, sel, 0).astype(np.int32)   # [4, n_need]
        needc = np.repeat(needp * 2, 2)
        needc[1::2] += 1                                    # both columns of pair
        rows = grows[np.repeat(gsel, 2, axis=1)]            # [4, 2n, 16]
        rows = rows.transpose(0, 2, 1).reshape(-1, needc.size)  # [64, 2n]
        subv = x[rows, needc[None, :]]
        vmask = np.repeat(np.repeat(selpass, 2, axis=1), GRP_ROWS, axis=0)
        subv = np.where(vmask, subv, np.float32(-1.0))
        new_cm = subv.max(axis=0)
        newach = subv == new_cm[None, :]
        new_ct = np.where(newach, rows, np.int64(10**6)).min(axis=0)
        better = new_cm > cm[needc]
        equal = new_cm == cm[needc]
        cm[needc] = np.where(better, new_cm, cm[needc])
        ct_n = ct[needc]
        ct[needc] = np.where(better, new_ct, np.where(equal, np.minimum(ct_n, new_ct), ct_n))

    smax = np.repeat(S_hi.max(axis=0), 2)
    ncand = np.zeros(mm, np.int64)                          # fallback only for smax==0
    bad = np.flatnonzero(smax <= 0)
    if bad.size:
        subb = x[:, bad]
        cm[bad] = subb.max(axis=0)
        ct[bad] = subb.argmax(axis=0)

    # ---- reference's segment/scatter logic --------------------------------
    jr = np.arange(n, dtype=np.int64)
    forced = np.full(m, -1, np.int64)
    np.maximum.at(forced, bp, jr)
    match = np.where(forced >= 0, forced, ct)

    forced2 = np.full(n, -1, np.int64)
    np.maximum.at(forced2, match, np.arange(m, dtype=np.int64))
    hit2 = np.bincount(match, minlength=n) > 0

    out = forced2.copy()
    need = np.where(~hit2)[0]
    for i in need:
        mask_i = np.count_nonzero((x[i] + EPS) >= cm)
        out[i] = bp[i] if mask_i > 0 else -1
    return out.astype(np.int32)


def kernel(x):
    import ml_dtypes

    x = np.ascontiguousarray(np.asarray(x, dtype=np.float32))
    e = encode(x)
    e_parts = []
    for c in range(NCORES):
        sh = np.zeros((N, M_PAD // 2), ml_dtypes.float8_e5m2)
        sh[:, :M_SH // 2] = e[:, c * (M_SH // 2):(c + 1) * (M_SH // 2)]
        e_parts.append(sh)
    wst = _make_wst()
    rbm_all, colg_all = _device_outputs(e_parts, wst)
    return _combine(x, rbm_all, colg_all)


# revision 15
# speedup vs baseline: 1.0840x; 1.0840x over previous
"""Bipartite matcher v4: fp8(e5m2) exp-code, pair-sorted packing, DoubleRow PE.

Device input per core: e8 = e5m2 code of 2^(KEXP*(x-1)/ln2)  [512, m_pad],
monotone nonnegative byte code of x (1 byte/elem -> DMA halves vs bf16).

Row side (DVE): reinterpret byte pairs as uint16. Lexicographic uint16 max of
packed pairs yields the exact max over EVEN columns in the high byte; a second
tree over (u<<8) yields the exact max over ODD columns. Both at 2x (2-byte TT).
Outputs rbmA/rbmB [128, n_chunks*nblk] uint16 per 512-column block.

Col side (PE): fp8e5 matmul against a shared block-diag ones stationary
[128, 32] (4-row slices), accumulated over the 4 row-chunks: colg group G =
sum of E over rows {4G..4G+4}+128c. Act copies PSUM->SBUF bf16.

Host recovery identical in structure to kernel2 (code-agnostic bounds):
  row: candidate blocks = blocks whose byte code max equals the row max
  col: groups with s_g >= s_max*0.99/16 contain the col argmax; all-zero
       columns (code underflow, ~5%) fall into the ncand>K full-scan path.
"""

import numpy as np

N = 512
M = 200000
NCORES = 8
M_SH = M // NCORES          # 25000
SUPER_W = 4096
M_PAD = 25088               # 6*4096 + 512
ROW_BLK = 512
NBLK = M_PAD // ROW_BLK     # 49
GRP_SL = 8                  # col-side row-slice height within a 128-chunk
NGRP = 128 // GRP_SL        # 16 groups; group G = rows {8G..8G+8} + 128c
GRP_ROWS = GRP_SL * 4       # 32 rows per group
PAD_VAL = 0.0
KEXP = 2000.0
EPS = np.float32(1e-12)

_CACHE: dict = {}


def _build_nc(m_pad=M_PAD, n_rows=N, loop_k=1):
    from concourse import bacc, mybir
    from concourse.tile import TileContext
    import concourse.bass as bass

    f8 = mybir.dt.float8e5
    u16 = mybir.dt.uint16
    bf16 = mybir.dt.bfloat16
    f32 = mybir.dt.float32
    n_chunks = n_rows // 128
    nblk = m_pad // ROW_BLK
    ngrp = NGRP

    nc = bacc.Bacc(None, target_bir_lowering=False)
    e_sh = nc.declare_dram_parameter("e_sh", [n_rows, m_pad], f8, isOutput=False)
    wst = nc.declare_dram_parameter("wst", [128, 2 * ngrp], f8, isOutput=False)
    if loop_k > 1:
        nc.declare_dram_parameter("k_tag", [1, loop_k], f32, isOutput=False)
    rbma = nc.declare_dram_parameter("rbma", [128, n_chunks * nblk], u16, isOutput=True)
    colg = nc.declare_dram_parameter("colg", [ngrp, m_pad // 2], bf16, isOutput=True)

    tiles = []
    base = 0
    while base < m_pad:
        w = min(SUPER_W, m_pad - base)
        tiles.append((base, w))
        base += w
    # smallest supertile first: engines start after ~256KB instead of ~2MB
    tiles.sort(key=lambda t: t[1])

    with TileContext(nc) as tc:
        with (
            tc.tile_pool(name="x", bufs=8) as xpool,
            tc.tile_pool(name="lvl", bufs=2) as lpool,
            tc.tile_pool(name="cg", bufs=2) as cgpool,
            tc.tile_pool(name="outs", bufs=1) as opool,
            tc.tile_pool(name="ps", bufs=2, space=bass.MemorySpace.PSUM) as pspool,
        ):
            rbma_t = opool.tile([128, n_chunks * nblk], u16, name="rbmat", tag="rbmat")
            wst_t = opool.tile([128, 2 * ngrp], f8, name="wst", tag="wst")
            nc.gpsimd.dma_start(out=wst_t[:], in_=wst[:, :])

            def row_tree(src16, B, out_t, b0, nb):
                """uint16 max tree: j 256 ->128->64->32 then reduce ->1."""
                u1 = lpool.tile([128, B * 128], u16, name="v1", tag="v1")
                s3 = src16.rearrange("p (B j) -> p B j", j=256)
                nc.vector.tensor_tensor(
                    out=u1[:].rearrange("p (B j) -> p B j", j=128),
                    in0=s3[:, :, 0:128], in1=s3[:, :, 128:256],
                    op=mybir.AluOpType.max,
                )
                u2 = lpool.tile([128, B * 64], u16, name="v2", tag="v2")
                u13 = u1[:].rearrange("p (B j) -> p B j", j=128)
                nc.vector.tensor_tensor(
                    out=u2[:].rearrange("p (B j) -> p B j", j=64),
                    in0=u13[:, :, 0:64], in1=u13[:, :, 64:128],
                    op=mybir.AluOpType.max,
                )
                u3 = lpool.tile([128, B * 32], u16, name="v3", tag="v3")
                u23 = u2[:].rearrange("p (B j) -> p B j", j=64)
                nc.vector.tensor_tensor(
                    out=u3[:].rearrange("p (B j) -> p B j", j=32),
                    in0=u23[:, :, 0:32], in1=u23[:, :, 32:64],
                    op=mybir.AluOpType.max,
                )
                u4 = lpool.tile([128, B * 16], u16, name="v4", tag="v4")
                u33 = u3[:].rearrange("p (B j) -> p B j", j=32)
                nc.vector.tensor_tensor(
                    out=u4[:].rearrange("p (B j) -> p B j", j=16),
                    in0=u33[:, :, 0:16], in1=u33[:, :, 16:32],
                    op=mybir.AluOpType.max,
                )
                rb3 = out_t[:].rearrange("p (c b) -> p c b", b=nblk)
                nc.vector.tensor_reduce(
                    out=rb3[:, :, b0 // ROW_BLK:b0 // ROW_BLK + nb],
                    in_=u4[:].rearrange("p (B j) -> p B j", j=16),
                    axis=mybir.AxisListType.X,
                    op=mybir.AluOpType.max,
                )

            def body():
                for (b0, w) in tiles:
                    nb = w // ROW_BLK
                    B = n_chunks * nb
                    xt = xpool.tile([128, n_chunks * w], f8, name="xt", tag="x")
                    for c in range(n_chunks):
                        nc.sync.dma_start(
                            out=xt[:, c * w:(c + 1) * w],
                            in_=e_sh[c * 128:(c + 1) * 128, b0:b0 + w],
                        )
                    x16 = xt[:].bitcast(mybir.dt.uint16)      # [128, n_chunks*w/2]
                    # pairs are host-sorted (max byte high): ONE lexicographic
                    # uint16 tree yields the exact 512-col block byte max
                    row_tree(x16, B, rbma_t, b0, nb)
                    # ---- PE col-side group sums (fp8), Act copies to SBUF
                    # host decode only reads the pair-max (odd) columns, so
                    # the PE sums just those via a stride-2 moving AP: half
                    # the matmul cols, half the copies, half the colg out
                    HB = ROW_BLK // 2       # 256 odd columns per 512-block
                    cgt = cgpool.tile([ngrp, w // 2], bf16, name="cgt", tag="cg")
                    PSW = 4
                    for pg in range(0, nb, PSW):
                        bw = min(PSW, nb - pg)
                        ps = pspool.tile([ngrp, PSW * HB], f32, name="ps", tag="ps")
                        x4o = xt[:].rearrange(
                            "p (c q two) -> p c q two", c=n_chunks, two=2
                        )
                        w3 = wst_t[:].rearrange("p (t g) -> p t g", t=2)
                        for blk in range(pg, pg + bw):
                            o = (blk - pg) * HB
                            for cp in range(n_chunks // 2):
                                nc.tensor.matmul(
                                    ps[:, o:o + HB],
                                    w3,
                                    x4o[:, 2 * cp:2 * cp + 2,
                                        blk * HB:(blk + 1) * HB, 1:2],
                                    start=(cp == 0),
                                    stop=(cp == n_chunks // 2 - 1),
                                    perf_mode=mybir.MatmulPerfMode.DoubleRow,
                                )
                        nc.scalar.copy(
                            out=cgt[:, pg * HB: pg * HB + bw * HB],
                            in_=ps[:, :bw * HB],
                        )
                    # outputs go via the idle GpSimd SWDGE queue so a copy-
                    # gated output DMA never head-of-line blocks input DMAs
                    nc.gpsimd.dma_start(
                        out=colg[:, b0 // 2:(b0 + w) // 2], in_=cgt[:]
                    )

            if loop_k == 1:
                body()
            else:
                with tc.For_i(0, loop_k, 1):
                    body()

            nc.gpsimd.dma_start(out=rbma[:, :], in_=rbma_t[:])
    nc.compile()
    return nc


def _make_wst():
    import ml_dtypes

    w = np.zeros((128, 2, NGRP), np.float32)
    for p in range(128):
        w[p, :, p // GRP_SL] = 1.0
    return w.reshape(128, 2 * NGRP).astype(ml_dtypes.float8_e5m2)


def _group_rows(n_rows=N):
    g = np.arange(NGRP)
    rows = (
        g[:, None, None] * GRP_SL
        + np.arange(GRP_SL)[None, :, None]
        + 128 * np.arange(n_rows // 128)[None, None, :]
    )
    return np.sort(rows.reshape(NGRP, -1), axis=1).astype(np.int32)  # [32, 16]


def encode(x):
    """e5m2 byte code ~ 2^(KEXP*(x-1)/ln2), built directly in code space.

    The e5m2 bit pattern ((e+15)<<2)|m is a log-linear code, so
    p = 4*(KEXP*(x-1)/ln2 + 15) + 0.5, clamped to [0, 255] and truncated, is a
    monotone nonnegative code of x (Schraudolph trick at 8-bit width). The
    candidate bounds only need monotonicity + nonnegativity, not exactness.
    """
    import ml_dtypes

    a8 = np.float32(4.0 * KEXP / np.log(2.0))
    z = (x.astype(np.float32) - np.float32(1.0)) * a8 + np.float32(60.5)
    np.maximum(z, np.float32(0.0), out=z)
    e = z.astype(np.uint8)
    # sort each adjacent column pair (max into the ODD byte = uint16 high
    # byte on little-endian): a single lexicographic uint16 max tree then
    # recovers the exact block byte max; PE group sums are order-invariant.
    a = e[:, 0::2]
    b = e[:, 1::2]
    e[:, 1::2] = np.maximum(a, b)
    e[:, 0::2] = np.minimum(a, b)
    return e.view(ml_dtypes.float8_e5m2)


def _get_nc():
    if "nc" not in _CACHE:
        _CACHE["nc"] = _build_nc()
    return _CACHE["nc"]


def _device_outputs(e_parts, wst):
    import os

    from concourse.bass_utils import run_bass_kernel_spmd

    in_maps = [{"e_sh": e_parts[c], "wst": wst} for c in range(NCORES)]
    try:
        bkr = run_bass_kernel_spmd(_get_nc(), in_maps, list(range(NCORES)))
    except ModuleNotFoundError:
        # profiling hook unavailable in this environment: run untraced
        os.environ["BASS_NEVER_TRACE"] = "1"
        bkr = run_bass_kernel_spmd(_get_nc(), in_maps, list(range(NCORES)))
    _CACHE["last_bkr"] = bkr
    res = bkr.results
    rbm_all = []
    colg_all = []
    for c in range(NCORES):
        ra = np.asarray(res[c]["rbma"]).view(np.uint16) >> np.uint16(8)
        rbm_all.append(ra.astype(np.uint8))  # byte code block max
        colg_all.append(np.asarray(res[c]["colg"]))
    return rbm_all, colg_all


def _combine(x, rbm_all, colg_all, cand_k=4):
    import ml_dtypes

    n, m = x.shape

    # ---- row side ---------------------------------------------------------
    rbm_full = np.concatenate(
        [
            rbm_all[k].astype(np.int16).reshape(128, 4, NBLK)
            .transpose(1, 0, 2).reshape(n, NBLK)
            for k in range(NCORES)
        ],
        axis=1,
    )  # [512, 8*NBLK] byte codes
    rmax = rbm_full.max(axis=1)
    bp = np.empty(n, np.int64)
    for i in range(n):
        cand = np.flatnonzero(rbm_full[i] == rmax[i])
        segs, idxs = [], []
        for gb in cand:
            core, blk = divmod(int(gb), NBLK)
            c0 = blk * ROW_BLK
            w = min(ROW_BLK, M_SH - c0)
            if w <= 0:
                continue
            g0 = core * M_SH + c0
            segs.append(x[i, g0:g0 + w])
            idxs.append(np.arange(g0, g0 + w))
        if not segs:  # degenerate: whole-row code underflow
            bp[i] = int(x[i].argmax())
            continue
        vals = np.concatenate(segs)
        cols = np.concatenate(idxs)
        bp[i] = cols[int(vals.argmax())]

    # ---- col side: iterative exact decode at pair granularity -------------
    # Device col 2k+1 holds pair-max codes; S_hi[g,k] = sum of 16 pair-max
    # codes. Any row r with x[r,m] > cm has group sum S_hi >= code(x[r,m])
    # >= code(cm), so gathering all ungathered groups meeting that bound and
    # repeating until none remain yields the exact col max + first argmax.
    S_hi = np.concatenate(
        [colg_all[k][:, :M_SH // 2].astype(np.float32) for k in range(NCORES)],
        axis=1,
    )  # [16, M/2]: device emits pair-max-column sums only
    grows = _group_rows()                                   # [32, 16] int32
    a8 = np.float32(4.0 * KEXP / np.log(2.0))
    c605 = np.float32(60.5)

    mm = m
    colidx = np.arange(mm, dtype=np.int64)
    gathered = np.zeros((NGRP, mm // 2), bool)              # per PAIR
    cm = np.full(mm, -1.0, np.float32)
    ct = np.full(mm, 10**6, np.int64)

    # pass 0: top-1 group by S_hi per pair
    g0 = S_hi.argmax(axis=0)                                # [M/2]
    gathered[g0, np.arange(mm // 2)] = True
    rows0 = grows[np.repeat(g0, 2)]                         # [M, 16]
    sub0 = x[rows0.T, colidx[None, :]]                      # [16, M]
    cm = sub0.max(axis=0)
    ach = sub0 == cm[None, :]
    ct = np.where(ach, rows0.T, np.int64(10**6)).min(axis=0)

    lut = np.arange(256, dtype=np.uint8).view(ml_dtypes.float8_e5m2).astype(
        np.float32
    )  # byte code -> e5m2 value (the domain the PE sums live in)
    for _ in range(32):
        t = (cm - np.float32(1.0)) * a8 + c605
        np.maximum(t, np.float32(0.0), out=t)
        tcode = lut[t.astype(np.uint8)]                     # value of code(cm)
        tpair = np.minimum(tcode[0::2], tcode[1::2])        # conservative per pair
        # 0.99: bf16-stored sums can sit ~2^-9 below the true sum
        passing = (S_hi >= (tpair * np.float32(0.99))[None, :]) & ~gathered
        npass = passing.sum(axis=0)
        needp = np.flatnonzero(npass > 0)
        if needp.size == 0:
            break
        # gather up to 4 passing groups per needy pair this round
        sel = np.argsort(~passing[:, needp], axis=0, kind="stable")[:4]  # passing first
        selpass = np.take_along_axis(passing[:, needp], sel, axis=0)
        gathered[sel, needp[None, :]] |= selpass
        gsel = np.where(selpass, sel, 0).astype(np.int32)   # [4, n_need]
        needc = np.repeat(needp * 2, 2)
        needc[1::2] += 1                                    # both columns of pair
        rows = grows[np.repeat(gsel, 2, axis=1)]            # [4, 2n, 16]
        rows = rows.transpose(0, 2, 1).reshape(-1, needc.size)  # [64, 2n]
        subv = x[rows, needc[None, :]]
        vmask = np.repeat(np.repeat(selpass, 2, axis=1), GRP_ROWS, axis=0)
        subv = np.where(vmask, subv, np.float32(-1.0))
        new_cm = subv.max(axis=0)
        newach = subv == new_cm[None, :]
        new_ct = np.where(newach, rows, np.int64(10**6)).min(axis=0)
        better = new_cm > cm[needc]
        equal = new_cm == cm[needc]
        cm[needc] = np.where(better, new_cm, cm[needc])
        ct_n = ct[needc]
        ct[needc] = np.where(better, new_ct, np.where(equal, np.minimum(ct_n, new_ct), ct_n))

    smax = np.repeat(S_hi.max(axis=0), 2)
    ncand = np.zeros(mm, np.int64)                          # fallback only for smax==0
    bad = np.flatnonzero(smax <= 0)
    if bad.size:
        subb = x[:, bad]
        cm[bad] = subb.max(axis=0)
        ct[bad] = subb.argmax(axis=0)

    # ---- reference's segment/scatter logic --------------------------------
    jr = np.arange(n, dtype=np.int64)
    forced = np.full(m, -1, np.int64)
    np.maximum.at(forced, bp, jr)
    match = np.where(forced >= 0, forced, ct)

    forced2 = np.full(n, -1, np.int64)
    np.maximum.at(forced2, match, np.arange(m, dtype=np.int64))
    hit2 = np.bincount(match, minlength=n) > 0

    out = forced2.copy()
    need = np.where(~hit2)[0]
    for i in need:
        mask_i = np.count_nonzero((x[i] + EPS) >= cm)
        out[i] = bp[i] if mask_i > 0 else -1
    return out.astype(np.int32)


def kernel(x):
    import ml_dtypes

    x = np.ascontiguousarray(np.asarray(x, dtype=np.float32))
    e = encode(x)
    e_parts = []
    for c in range(NCORES):
        sh = np.zeros((N, M_PAD), ml_dtypes.float8_e5m2)
        sh[:, :M_SH] = e[:, c * M_SH:(c + 1) * M_SH]
        e_parts.append(sh)
    wst = _make_wst()
    rbm_all, colg_all = _device_outputs(e_parts, wst)
    return _combine(x, rbm_all, colg_all)
